# revision 1
# baseline (speedup 1.0000x reference)
"""BoxFilter kernel for Trainium2 (8 NeuronCores).

Computes out[b,0,i,j] = sum_{c} sum_{|di|<=15} sum_{|dj|<=15} x[b,c,i+di,j+dj]
(edge-clamped 31x31 box filter over the channel-summed image), matching the
reference cumsum + shifted-diff formulation exactly (separable box sums).

Sharding: data-parallel over (batch, H-half) -> 8 shards, no cross-core
communication. Each core receives a host-padded [3, 1056, 2048] slab
(16 halo rows on each side, zero-filled past the global image edges).

Per-core pipeline (all f32):
  1. channel-sum on DVE (2 adds per 128-row tile)
  2. vertical 31-tap box sum via two banded 0/1-matrix fp32 matmuls per
     PSUM bank (bands are compile-time constant inputs)
  3. ACT copies PSUM -> zero-padded SBUF tile
  4. horizontal 31-tap box sum in a single tensor_tensor_scan:
     state_j = state_{j-1} + xp[j] - xp[j-31]
  5. DMA result rows to DRAM
"""

import numpy as np

R = 15
TAP = 2 * R + 1          # 31
B, C, H, W = 4, 3, 2048, 2048
HALF = H // 2            # 1024 output rows per core
S_ROWS = HALF + 32       # 1056 input rows per core (16-row halo each side)
N_CORES = 8
PAD_L = TAP              # left zero pad for the scan (31)
PAD_R = R                # right zero pad (15)
XP_W = PAD_L + W + PAD_R # 2094
SCAN_N = W + R           # 2063 scan steps; out col j = scan[j + R]
P = 128                  # SBUF partitions
N_OUT_TILES = HALF // P  # 8
TAIL_ROWS = S_ROWS - N_OUT_TILES * P  # 32 valid rows in the 9th s-tile
MM_N = 512               # fp32 moving-operand max / one PSUM bank

_CACHE = {}


def _band_matrices():
    # out row i of a 128-row tile needs halo'd input rows r = i+1 .. i+31
    # (r is the row index within the [s_lo; s_hi] 256-row window).
    k = np.arange(P)[:, None]
    i = np.arange(P)[None, :]
    band_a = ((k >= i + 1) & (k <= i + TAP)).astype(np.float32)          # rows in s_lo
    band_b = ((k + P >= i + 1) & (k + P <= i + TAP)).astype(np.float32)  # rows in s_hi
    return band_a, band_b


def _build_kernel(tc, nc, out, xs, band_a_d, band_b_d, mybir, bass):
    from contextlib import ExitStack

    f32 = mybir.dt.float32
    f32r = mybir.dt.float32r
    add = mybir.AluOpType.add
    sub = mybir.AluOpType.subtract

    with ExitStack() as ctx:
        const_pool = ctx.enter_context(tc.tile_pool(name="const", bufs=1))
        s_pool = ctx.enter_context(tc.tile_pool(name="s", bufs=4))
        xp_pool = ctx.enter_context(tc.tile_pool(name="xp", bufs=2))
        box_pool = ctx.enter_context(tc.tile_pool(name="box", bufs=2))
        psum_pool = ctx.enter_context(
            tc.tile_pool(name="psum", bufs=8, space=bass.MemorySpace.PSUM)
        )

        xc_pool = ctx.enter_context(tc.tile_pool(name="xc", bufs=4))

        band_a = const_pool.tile([P, P], f32r)
        band_b = const_pool.tile([P, P], f32r)
        nc.sync.dma_start(band_a[:], band_a_d)
        nc.sync.dma_start(band_b[:], band_b_d)

        def make_s(u):
            rows = P if u < N_OUT_TILES else TAIL_ROWS
            s = s_pool.tile([P, W], f32r)
            if rows < P:
                # rows >= TAIL_ROWS are multiplied by zero band weights but
                # must be finite, and rows 31.. are simply past the image.
                nc.gpsimd.memset(s[:].bitcast(f32), 0.0)
            if u < 2:
                # pipeline-fill fast path: the first matmul needs s_0 AND
                # s_1, so land them ASAP — one 1MB DMA per channel spread
                # over all three DMA lanes (sync HWDGE, scalar HWDGE,
                # gpsimd SWDGE), adds on the fast engine (DVE).
                xc = xc_pool.tile([P, C, W], f32)
                for c, eng in ((0, nc.sync), (1, nc.scalar), (2, nc.gpsimd)):
                    eng.dma_start(
                        xc[:rows, c, :], xs[c, P * u : P * u + rows, :]
                    )
                nc.vector.tensor_add(s[:rows, :], xc[:rows, 0, :], xc[:rows, 1, :])
                nc.vector.tensor_add(s[:rows, :], s[:rows, :], xc[:rows, 2, :])
                return s
            # steady state: one batched DMA for all 3 channels: [rows, 3, W],
            # partition-major, alternating HWDGE rings (sync vs scalar) — a
            # single logical DMA queue tops out well below per-core HBM
            # bandwidth.
            xc = xc_pool.tile([P, C, W], f32)
            dma_eng = nc.sync if u % 2 == 0 else nc.scalar
            dma_eng.dma_start(
                xc[:rows],
                xs[:, P * u : P * u + rows, :].rearrange("c p n -> p c n"),
            )
            # split the 2-input adds between DVE and GpSimd so neither
            # engine becomes the pipeline gate (GpSimd TT is ~2x slower)
            eng = nc.vector if u % 2 == 0 else nc.gpsimd
            eng.tensor_add(s[:rows, :], xc[:rows, 0, :], xc[:rows, 1, :])
            eng.tensor_add(s[:rows, :], s[:rows, :], xc[:rows, 2, :])
            return s

        s_tiles = {0: make_s(0)}
        for t in range(N_OUT_TILES):
            s_tiles[t + 1] = make_s(t + 1)
            s_lo, s_hi = s_tiles.pop(t), s_tiles[t + 1]

            xp = xp_pool.tile([P, XP_W], f32)
            nc.gpsimd.memset(xp[:, 0:PAD_L], 0.0)
            nc.gpsimd.memset(xp[:, PAD_L + W : XP_W], 0.0)

            # all band_a matmuls, then all band_b: minimizes PE weight reloads
            psums = []
            for nb in range(W // MM_N):
                ps = psum_pool.tile([P, MM_N], f32)
                lo_c = s_lo[:, MM_N * nb : MM_N * (nb + 1)]
                nc.tensor.matmul(
                    ps[:], band_a[:], lo_c, start=True, stop=False
                )
                psums.append(ps)
            for nb in range(W // MM_N):
                hi_c = s_hi[:, MM_N * nb : MM_N * (nb + 1)]
                nc.tensor.matmul(
                    psums[nb][:], band_b[:], hi_c,
                    start=False, stop=True,
                )
                nc.scalar.copy(
                    xp[:, PAD_L + MM_N * nb : PAD_L + MM_N * (nb + 1)],
                    psums[nb][:],
                )

            box = box_pool.tile([P, SCAN_N + 1], f32)
            nc.vector.tensor_tensor_scan(
                box[:, 0:SCAN_N],
                xp[:, PAD_L : PAD_L + SCAN_N],
                xp[:, 0:SCAN_N],
                0.0,
                add,
                sub,
            )
            store_eng = nc.scalar if t % 2 == 0 else nc.sync
            store_eng.dma_start(out[P * t : P * (t + 1), :], box[:, R : R + W])


def _get_nc():
    if "nc" in _CACHE:
        return _CACHE["nc"]
    import concourse.bass as bass
    import concourse.tile as tile
    from concourse import bacc, mybir

    nc = bacc.Bacc(
        "TRN2", target_bir_lowering=False, debug=False, num_devices=N_CORES
    )
    xs = nc.dram_tensor("xs", [C, S_ROWS, W], mybir.dt.float32, kind="ExternalInput")
    ba = nc.dram_tensor("band_a", [P, P], mybir.dt.float32r, kind="ExternalInput")
    bb = nc.dram_tensor("band_b", [P, P], mybir.dt.float32r, kind="ExternalInput")
    out = nc.dram_tensor("out", [HALF, W], mybir.dt.float32, kind="ExternalOutput")

    with tile.TileContext(nc) as tc:
        _build_kernel(tc, nc, out.ap(), xs.ap(), ba.ap(), bb.ap(), mybir, bass)
    nc.compile()
    _CACHE["nc"] = nc
    return nc


def _in_maps(x):
    band_a, band_b = _band_matrices()
    maps = []
    for k in range(N_CORES):
        b, half = divmod(k, 2)
        h0 = half * HALF
        lo = h0 - 16  # global row of xs row 0
        g0, g1 = max(lo, 0), min(h0 + HALF + 16, H)
        xs = np.zeros((C, S_ROWS, W), np.float32)
        xs[:, g0 - lo : g1 - lo, :] = x[b, :, g0:g1, :]
        maps.append({"xs": xs, "band_a": band_a, "band_b": band_b})
    return maps


def _run(x, trace=False, tmpdir=None):
    from concourse.bass_utils import run_bass_kernel_spmd

    nc = _get_nc()
    res = run_bass_kernel_spmd(
        nc, _in_maps(x), list(range(N_CORES)), trace=trace, tmpdir=tmpdir
    )
    out = np.empty((B, 1, H, W), np.float32)
    for k in range(N_CORES):
        b, half = divmod(k, 2)
        out[b, 0, half * HALF : (half + 1) * HALF, :] = res.results[k]["out"]
    return out, res


def kernel(x: np.ndarray) -> np.ndarray:
    x = np.ascontiguousarray(x, dtype=np.float32)
    assert x.shape == (B, C, H, W)
    return _run(x)[0]



# revision 3
# speedup vs baseline: 1.2333x; 1.2333x over previous
"""BoxFilter kernel for Trainium2 (8 NeuronCores), bf16 I/O.

Computes out[b,0,i,j] = sum_{c} sum_{|di|<=15} sum_{|dj|<=15} x[b,c,i+di,j+dj]
(edge-clamped 31x31 box filter over the channel-summed image), matching the
reference cumsum + shifted-diff formulation (separable box sums).

The correctness gate is rel_err < 2e-2; bf16 end-to-end measures ~5e-3, so
all HBM traffic runs in bf16 (half the bytes of the f32 baseline):
host casts x to bf16 and interleaves channels per row ([S_ROWS, C, W] per
core), the device computes with bf16 SBUF tiles + f32 PSUM/scan state, and
the output returns as bf16 which the host upcasts.

Sharding: data-parallel over (batch, H-half) -> 8 shards, no cross-core
communication. Each core gets a host-padded [1056, 3, 2048] slab (16 halo
rows each side, zero-filled past the global image edges).

Per-core pipeline (per 128-row output tile):
  1. one contiguous 1.5MB DMA -> xc[128, 3, 2048] (HWDGE, alternating rings)
  2. channel sum: 2 tensor_tensor adds, bf16 (DVE 2x-packed) or GpSimd,
     split per-tile to balance the engines
  3. vertical 31-tap box sum: two banded 0/1 bf16 matmuls per PSUM bank
  4. ACT copies PSUM f32 -> bf16 xp tile (pads pre-zeroed once)
  5. horizontal 31-tap box sum: DVE tensor_tensor_scan (fp32 state,
     bf16 in/out): state_j = state_{j-1} + xp[j] - xp[j-31]
  6. DMA box rows (bf16) to DRAM
"""

import numpy as np
import ml_dtypes

BF16 = ml_dtypes.bfloat16

R = 15
TAP = 2 * R + 1          # 31
B, C, H, W = 4, 3, 2048, 2048
HALF = H // 2            # 1024 output rows per core
S_ROWS = HALF + 32       # 1056 input rows per core (16-row halo each side)
N_CORES = 8
PAD_L = TAP              # left zero pad for the scan (31)
PAD_R = R                # right zero pad (15)
XP_W = PAD_L + W + PAD_R # 2094
SCAN_N = W + R           # 2063 scan steps; out col j = scan[j + R]
P = 128                  # SBUF partitions
N_OUT_TILES = HALF // P  # 8
TAIL_ROWS = S_ROWS - N_OUT_TILES * P  # 32 valid rows in the 9th s-tile
MM_N = 512               # one PSUM bank of f32

# per s-tile engine for the two channel-sum adds: v=DVE (fast, but owns the
# scans), g=GpSimd (slower, otherwise idle)
ADD_ENG = "vvgvgvgvg"

_CACHE = {}


def _band_matrices():
    # out row i of a 128-row tile needs halo'd input rows r = i+1 .. i+31
    # (r is the row index within the [s_lo; s_hi] 256-row window).
    k = np.arange(P)[:, None]
    i = np.arange(P)[None, :]
    band_a = ((k >= i + 1) & (k <= i + TAP)).astype(BF16)          # rows in s_lo
    band_b = ((k + P >= i + 1) & (k + P <= i + TAP)).astype(BF16)  # rows in s_hi
    return band_a, band_b


def _build_kernel(tc, nc, out, xs, band_a_d, band_b_d, mybir, bass):
    from contextlib import ExitStack

    bf = mybir.dt.bfloat16
    f32 = mybir.dt.float32
    add = mybir.AluOpType.add
    sub = mybir.AluOpType.subtract

    with ExitStack() as ctx:
        const_pool = ctx.enter_context(tc.tile_pool(name="const", bufs=1))
        xc_pool = ctx.enter_context(tc.tile_pool(name="xc", bufs=6))
        s_pool = ctx.enter_context(tc.tile_pool(name="s", bufs=4))
        xp_pool = ctx.enter_context(tc.tile_pool(name="xp", bufs=3))
        box_pool = ctx.enter_context(tc.tile_pool(name="box", bufs=3))
        psum_pool = ctx.enter_context(
            tc.tile_pool(name="psum", bufs=8, space=bass.MemorySpace.PSUM)
        )

        band_a = const_pool.tile([P, P], bf)
        band_b = const_pool.tile([P, P], bf)
        nc.sync.dma_start(band_a[:], band_a_d)
        nc.sync.dma_start(band_b[:], band_b_d)

        # xp buffers: zero the scan pads once; ACT only ever writes the
        # middle [PAD_L, PAD_L+W) region.
        xp_tiles = [
            xp_pool.tile([P, XP_W], bf, name=f"xp{i}") for i in range(3)
        ]
        for xp in xp_tiles:
            nc.gpsimd.memset(xp[:, 0:PAD_L], 0.0)
            nc.gpsimd.memset(xp[:, PAD_L + W : XP_W], 0.0)

        def make_s(u):
            rows = P if u < N_OUT_TILES else TAIL_ROWS
            s = s_pool.tile([P, W], bf)
            if rows < P:
                # rows past the slab are multiplied by zero band weights but
                # must be finite.
                nc.gpsimd.memset(s[:], 0.0)
            xc = xc_pool.tile([P, C, W], bf)
            if u < 2:
                # pipeline-fill fast path: the first matmul needs s_0 AND
                # s_1 - halve each load's latency by splitting it across
                # both HWDGE rings.
                nc.sync.dma_start(xc[0:64], xs[P * u : P * u + 64])
                nc.scalar.dma_start(xc[64:P], xs[P * u + 64 : P * (u + 1)])
            else:
                eng = nc.sync if u % 2 == 0 else nc.scalar
                eng.dma_start(xc[:rows], xs[P * u : P * u + rows])
            aeng = nc.vector if ADD_ENG[u] == "v" else nc.gpsimd
            aeng.tensor_add(s[:rows, :], xc[:rows, 0, :], xc[:rows, 1, :])
            aeng.tensor_add(s[:rows, :], s[:rows, :], xc[:rows, 2, :])
            return s

        s_tiles = {0: make_s(0)}
        for t in range(N_OUT_TILES):
            s_tiles[t + 1] = make_s(t + 1)
            s_lo, s_hi = s_tiles.pop(t), s_tiles[t + 1]

            xp = xp_tiles[t % 3]

            # all band_a matmuls, then all band_b: minimizes PE weight reloads
            psums = []
            for nb in range(W // MM_N):
                ps = psum_pool.tile([P, MM_N], f32)
                lo_c = s_lo[:, MM_N * nb : MM_N * (nb + 1)]
                nc.tensor.matmul(ps[:], band_a[:], lo_c, start=True, stop=False)
                psums.append(ps)
            for nb in range(W // MM_N):
                hi_c = s_hi[:, MM_N * nb : MM_N * (nb + 1)]
                nc.tensor.matmul(
                    psums[nb][:], band_b[:], hi_c, start=False, stop=True
                )
                nc.scalar.copy(
                    xp[:, PAD_L + MM_N * nb : PAD_L + MM_N * (nb + 1)],
                    psums[nb][:],
                )

            box = box_pool.tile([P, SCAN_N + 1], bf)
            nc.vector.tensor_tensor_scan(
                box[:, 0:SCAN_N],
                xp[:, PAD_L : PAD_L + SCAN_N],
                xp[:, 0:SCAN_N],
                0.0,
                add,
                sub,
            )
            store_eng = nc.scalar if t % 2 == 0 else nc.sync
            store_eng.dma_start(out[P * t : P * (t + 1), :], box[:, R : R + W])


def _get_nc():
    if "nc" in _CACHE:
        return _CACHE["nc"]
    import concourse.bass as bass
    import concourse.tile as tile
    from concourse import bacc, mybir

    nc = bacc.Bacc(
        "TRN2", target_bir_lowering=False, debug=False, num_devices=N_CORES
    )
    xs = nc.dram_tensor("xs", [S_ROWS, C, W], mybir.dt.bfloat16, kind="ExternalInput")
    ba = nc.dram_tensor("band_a", [P, P], mybir.dt.bfloat16, kind="ExternalInput")
    bb = nc.dram_tensor("band_b", [P, P], mybir.dt.bfloat16, kind="ExternalInput")
    out = nc.dram_tensor("out", [HALF, W], mybir.dt.bfloat16, kind="ExternalOutput")

    with tile.TileContext(nc) as tc:
        _build_kernel(tc, nc, out.ap(), xs.ap(), ba.ap(), bb.ap(), mybir, bass)
    nc.compile()
    _CACHE["nc"] = nc
    return nc


def _in_maps(x):
    band_a, band_b = _band_matrices()
    xb = x.astype(BF16)
    maps = []
    for k in range(N_CORES):
        b, half = divmod(k, 2)
        h0 = half * HALF
        lo = h0 - 16  # global row of xs row 0
        g0, g1 = max(lo, 0), min(h0 + HALF + 16, H)
        xs = np.zeros((S_ROWS, C, W), BF16)
        # [C, rows, W] -> [rows, C, W]
        xs[g0 - lo : g1 - lo] = xb[b, :, g0:g1, :].transpose(1, 0, 2)
        maps.append({"xs": xs, "band_a": band_a, "band_b": band_b})
    return maps


def _run(x, trace=False, tmpdir=None):
    from concourse.bass_utils import run_bass_kernel_spmd

    nc = _get_nc()
    res = run_bass_kernel_spmd(
        nc, _in_maps(x), list(range(N_CORES)), trace=trace, tmpdir=tmpdir
    )
    out = np.empty((B, 1, H, W), np.float32)
    for k in range(N_CORES):
        b, half = divmod(k, 2)
        out[b, 0, half * HALF : (half + 1) * HALF, :] = (
            res.results[k]["out"].astype(np.float32)
        )
    return out, res


def kernel(x: np.ndarray) -> np.ndarray:
    x = np.ascontiguousarray(x, dtype=np.float32)
    assert x.shape == (B, C, H, W)
    return _run(x)[0]
